# revision 16
# baseline (speedup 1.0000x reference)
"""Bahdanau-style attention kernel for 8 Trainium2 NeuronCores.

Computes (per reference):
    att1 = enc @ We + We_b          (B, P, A)
    att2 = dh @ Wd + Wd_b           (B, 1, A)
    h    = relu(att1 + att2)
    f    = h @ Wf + Wf_b            (B, P)
    alpha= softmax(f, axis=P)
    awe  = einsum('bpe,bp->be', enc, alpha)

Sharding: data-parallel over batch, 32 batches per core, weights replicated.
The host pre-transposes each encoder shard to (pair, E, 2, P) bf16 so that E
lands on SBUF partitions (contraction dim of the big matmul) and each DMA row
is 784B contiguous.  Batches are processed in pairs (moving free dim 392).

Per pair: PE accumulates att1 over 16 E-chunks in PSUM; ACT applies
relu(att1 + att2 + biases) with a per-partition bias; PE computes the logits
with a k-partitioned matvec; softmax runs on DVE/ACT (Exp with fused
accumulation); alpha is broadcast across partitions by GPSIMD; and
awe = sum_p alpha * enc runs on DVE (bf16 multiply + reduce, fp32
accumulation), with a few chunks' reductions offloaded to ACT.
All stages are software-pipelined across pairs so PE never waits on the
softmax chain.  Wf_b is dropped: softmax is shift-invariant.
"""

import sys
from contextlib import ExitStack

import ml_dtypes
import numpy as np

if "/opt/trn_rl_repo" not in sys.path:
    sys.path.insert(0, "/opt/trn_rl_repo")

import concourse.bass as bass  # noqa: F401  (bass types via bacc)
import concourse.tile as tile
from concourse import bacc, mybir
from concourse.bass import ts
from concourse.bass_utils import run_bass_kernel_spmd

N_CORES = 8
B, P, E, D, A = 256, 196, 2048, 512, 512
BL = B // N_CORES          # 32 batches per core
KO = E // 128              # 16 e-chunks
AC = A // 128              # 4 a-chunks
DC = D // 128              # 4 d-chunks
W = 2 * P                  # 392: moving free dim for a batch-pair
NPAIR = BL // 2            # 16
ACT_KOS = 2                # e-chunks whose awe-reduce runs on ACT

F32 = mybir.dt.float32
BF16 = mybir.dt.bfloat16
AF = mybir.ActivationFunctionType
OP = mybir.AluOpType


def build_kernel(repeat=1):
    nc = bacc.Bacc(
        "TRN2",
        target_bir_lowering=False,
        debug=False,
        enable_asserts=False,
        num_devices=N_CORES,
    )
    enc_t = nc.dram_tensor("enc_t", (NPAIR, E, 2, P), BF16, kind="ExternalInput").ap()
    dh_t = nc.dram_tensor("dh_t", (D, BL), BF16, kind="ExternalInput").ap()
    we = nc.dram_tensor("we", (E, A), BF16, kind="ExternalInput").ap()
    wd = nc.dram_tensor("wd", (D, A), BF16, kind="ExternalInput").ap()
    web = nc.dram_tensor("web", (A,), F32, kind="ExternalInput").ap()
    wdb = nc.dram_tensor("wdb", (A,), F32, kind="ExternalInput").ap()
    wf = nc.dram_tensor("wf", (A,), BF16, kind="ExternalInput").ap()
    awe_t = nc.dram_tensor("awe_t", (E, BL), F32, kind="ExternalOutput").ap()
    alpha_o = nc.dram_tensor("alpha_o", (1, BL, P), F32, kind="ExternalOutput").ap()

    with tile.TileContext(nc) as tc, ExitStack() as ctx:
        _emit(ctx, tc, enc_t, dh_t, we, wd, web, wdb, wf, awe_t, alpha_o, repeat)
    nc.compile()
    return nc


def _emit(ctx, tc, enc_t, dh_t, we, wd, web, wdb, wf, awe_t, alpha_o, repeat=1):
    nc = tc.nc

    cpool = ctx.enter_context(tc.tile_pool(name="consts", bufs=1))
    encb_pool = ctx.enter_context(tc.tile_pool(name="encb", bufs=4))
    h_pool = ctx.enter_context(tc.tile_pool(name="h", bufs=2))
    ab_pool = ctx.enter_context(tc.tile_pool(name="ab", bufs=3))
    scr_pool = ctx.enter_context(tc.tile_pool(name="scr", bufs=4))
    sm_pool = ctx.enter_context(tc.tile_pool(name="sm", bufs=2))
    psA = ctx.enter_context(tc.tile_pool(name="psA", bufs=4, space="PSUM"))
    psF = ctx.enter_context(tc.tile_pool(name="psF", bufs=2, space="PSUM"))

    # ---- resident constants (loaded once; the big We load is interleaved
    # with the first pair's encoder DMAs inside the loop below) --------------
    wd_sb = cpool.tile([128, DC, A], BF16, tag="wd")
    nc.sync.dma_start(wd_sb[:], wd.rearrange("(k p) a -> p k a", p=128))
    dh_sb = cpool.tile([128, DC, BL], BF16, tag="dh")
    nc.sync.dma_start(dh_sb[:], dh_t.rearrange("(k p) b -> p k b", p=128))
    wf_sb = cpool.tile([128, AC], BF16, tag="wf")
    nc.sync.dma_start(wf_sb[:], wf.rearrange("(k p) -> p k", p=128))
    web_sb = cpool.tile([128, AC], F32, tag="web")
    nc.sync.dma_start(web_sb[:], web.rearrange("(k p) -> p k", p=128))
    wdb_sb = cpool.tile([128, AC], F32, tag="wdb")
    nc.sync.dma_start(wdb_sb[:], wdb.rearrange("(k p) -> p k", p=128))
    we_sb = cpool.tile([128, KO, A], BF16, tag="we")

    bias_sb = cpool.tile([128, AC], F32, tag="bias")
    nc.vector.tensor_add(bias_sb[:], web_sb[:], wdb_sb[:])

    alpha_sb = cpool.tile([1, BL, P], F32, tag="alpha")
    awe_sb = cpool.tile([128, KO, BL], F32, tag="awe")
    att12_sb = cpool.tile([128, AC, BL], F32, tag="att12")

    # ---- att2 = dh @ Wd + (Wd_b + We_b), laid out [a_part, a_chunk, b] -----
    for ac in range(AC):
        ps2 = psA.tile([128, W], F32, tag="att1")
        for kd in range(DC):
            nc.tensor.matmul(
                ps2[:, :BL],
                lhsT=wd_sb[:, kd, ts(ac, 128)],
                rhs=dh_sb[:, kd, :],
                start=(kd == 0),
                stop=(kd == DC - 1),
            )
        nc.vector.tensor_tensor(
            att12_sb[:, ac, :],
            ps2[:, :BL],
            bias_sb[:, ac, None].to_broadcast((128, BL)),
            OP.add,
        )

    abs_ = {}

    def tail_head(state):
        """f = Wf . h ; softmax -> alpha_sb; broadcast alpha (pair i)."""
        i, encB, hT = state
        fp = psF.tile([1, W], F32, tag="f")
        for ac in range(AC):
            nc.tensor.matmul(
                fp[:],
                lhsT=wf_sb[:, ac : ac + 1],
                rhs=hT[:, ac, :],
                start=(ac == 0),
                stop=(ac == AC - 1),
            )
        nfmax = sm_pool.tile([1, 2], F32, tag="fmax")
        nc.vector.tensor_reduce(
            nfmax[:],
            fp.rearrange("o (b q) -> o b q", b=2),
            mybir.AxisListType.X,
            OP.max,
            negate=True,
        )
        expf = sm_pool.tile([1, 2, P], F32, tag="expf")
        sume = sm_pool.tile([1, 2], F32, tag="sume")
        for j in range(2):
            nc.scalar.activation(
                expf[:, j, :],
                fp[:, j * P : (j + 1) * P],
                AF.Exp,
                bias=nfmax[:, j : j + 1],
                scale=1.0,
                accum_out=sume[:, j : j + 1],
            )
        rsum = sm_pool.tile([1, 2], F32, tag="rsum")
        nc.vector.reciprocal(rsum[:], sume[:])
        for j in range(2):
            nc.vector.tensor_scalar_mul(
                alpha_sb[:, 2 * i + j, :], expf[:, j, :], rsum[:, j : j + 1]
            )
        alb = sm_pool.tile([1, W], BF16, tag="alb")
        nc.vector.tensor_copy(
            alb[:],
            alpha_sb[:, 2 * i : 2 * i + 2, :].rearrange("o b q -> o (b q)"),
        )
        ab = ab_pool.tile([128, W], BF16, tag="ab")
        nc.gpsimd.partition_broadcast(ab[:], alb[:])
        abs_[i] = ab

    def tail_awe(state, act_kos=ACT_KOS, last=False):
        """awe[:, :, pair] = sum_p encB * alpha (bf16, fp32 accumulation).

        DVE does the multiplies (and most reductions); act_kos chunks'
        reductions go to ACT.  For the flush pair, half the multiplies run
        on the otherwise-idle GPSIMD to shorten the drain."""
        i, encB, hT = state
        ab = abs_.pop(i)
        for ko in range(KO):
            tmp = scr_pool.tile([128, W], BF16, tag="tmp")
            mul_eng = nc.gpsimd if (last and ko % 2 == 1) else nc.vector
            mul_eng.tensor_tensor(tmp[:], encB[:, ko, :], ab[:], OP.mult)
            if ko < act_kos:
                for j in range(2):
                    scr = scr_pool.tile([128, P], BF16, tag="scr")
                    nc.scalar.activation(
                        scr[:],
                        tmp[:, j * P : (j + 1) * P],
                        AF.Copy,
                        bias=0.0,
                        scale=1.0,
                        accum_out=awe_sb[:, ko, 2 * i + j : 2 * i + j + 1],
                    )
            else:
                nc.vector.tensor_reduce(
                    awe_sb[:, ko, 2 * i : 2 * i + 2],
                    tmp.rearrange("p (b q) -> p b q", b=2),
                    mybir.AxisListType.X,
                    OP.add,
                )

    for _rep in range(repeat):
        prev = None
        for i in range(NPAIR):
            encB = encb_pool.tile([128, KO, W], BF16, tag="encB")
            for q in range(4):
                nc.sync.dma_start(
                    encB[:, ts(q, 4), :],
                    enc_t[i, ts(q, 512), :, :].rearrange(
                        "(k p) j q -> p k (j q)", p=128
                    ),
                )
                if _rep == 0 and i == 0:
                    nc.sync.dma_start(
                        we_sb[:, ts(q, 4), :],
                        we[ts(q, 512), :].rearrange("(k p) a -> p k a", p=128),
                    )
            if prev is not None:
                tail_head(prev)

            hT = h_pool.tile([128, AC, W], BF16, tag="h")
            for ac in range(AC):
                ps = psA.tile([128, W], F32, tag="att1")
                for ko in range(KO):
                    nc.tensor.matmul(
                        ps[:],
                        lhsT=we_sb[:, ko, ts(ac, 128)],
                        rhs=encB[:, ko, :],
                        start=(ko == 0),
                        stop=(ko == KO - 1),
                    )
                for j in range(2):
                    nc.scalar.activation(
                        hT[:, ac, j * P : (j + 1) * P],
                        ps[:, j * P : (j + 1) * P],
                        AF.Relu,
                        bias=att12_sb[:, ac, 2 * i + j : 2 * i + j + 1],
                        scale=1.0,
                    )
            if prev is not None:
                tail_awe(prev)
            prev = (i, encB, hT)

        tail_head(prev)
        tail_awe(prev, act_kos=10, last=True)

        # ---- outputs -------------------------------------------------------
        nc.sync.dma_start(awe_t.rearrange("(k p) b -> p k b", p=128), awe_sb[:])
        nc.sync.dma_start(alpha_o, alpha_sb[:])


_NC_CACHE = {}


def kernel(encoder_out, decoder_hidden, We_w, We_b, Wd_w, Wd_b, Wf_w, Wf_b):
    encoder_out = np.asarray(encoder_out, dtype=np.float32)
    decoder_hidden = np.asarray(decoder_hidden, dtype=np.float32)
    bf = ml_dtypes.bfloat16
    We_bf = np.ascontiguousarray(np.asarray(We_w, dtype=np.float32).astype(bf))
    Wd_bf = np.ascontiguousarray(np.asarray(Wd_w, dtype=np.float32).astype(bf))
    Wf_bf = np.ascontiguousarray(np.asarray(Wf_w, dtype=np.float32).astype(bf))
    We_b = np.ascontiguousarray(np.asarray(We_b, dtype=np.float32))
    Wd_b = np.ascontiguousarray(np.asarray(Wd_b, dtype=np.float32))
    del Wf_b  # softmax is shift-invariant; Wf_b cancels in both outputs

    if "nc" not in _NC_CACHE:
        _NC_CACHE["nc"] = build_kernel()
    nc = _NC_CACHE["nc"]

    in_maps = []
    for c in range(N_CORES):
        sl = slice(c * BL, (c + 1) * BL)
        enc_tp = np.ascontiguousarray(
            encoder_out[sl]
            .reshape(NPAIR, 2, P, E)
            .transpose(0, 3, 1, 2)
            .astype(bf)
        )
        in_maps.append(
            {
                "enc_t": enc_tp,
                "dh_t": np.ascontiguousarray(decoder_hidden[0, sl].T.astype(bf)),
                "we": We_bf,
                "wd": Wd_bf,
                "web": We_b,
                "wdb": Wd_b,
                "wf": Wf_bf,
            }
        )

    res = run_bass_kernel_spmd(nc, in_maps, core_ids=list(range(N_CORES)))
    awe = np.concatenate([r["awe_t"].T for r in res.results], axis=0)
    alpha = np.concatenate([r["alpha_o"][0] for r in res.results], axis=0)
    return awe, alpha


# revision 17
# speedup vs baseline: 1.0590x; 1.0590x over previous
"""Bahdanau-style attention kernel for 8 Trainium2 NeuronCores.

Computes (per reference):
    att1 = enc @ We + We_b          (B, P, A)
    att2 = dh @ Wd + Wd_b           (B, 1, A)
    h    = relu(att1 + att2)
    f    = h @ Wf + Wf_b            (B, P)
    alpha= softmax(f, axis=P)
    awe  = einsum('bpe,bp->be', enc, alpha)

Sharding: data-parallel over batch, 32 batches per core, weights replicated.
The host pre-transposes each encoder shard to (pair, E, 2, P) bf16 so that E
lands on SBUF partitions (contraction dim of the big matmul) and each DMA row
is 784B contiguous.  Batches are processed in pairs (moving free dim 392).

Per pair: PE accumulates att1 over 16 E-chunks in PSUM; ACT applies
relu(att1 + att2 + biases) with a per-partition bias; PE computes the logits
with a k-partitioned matvec; softmax runs on DVE/ACT (Exp with fused
accumulation); alpha is broadcast across partitions by GPSIMD; and
awe = sum_p alpha * enc runs on DVE (bf16 multiply + reduce, fp32
accumulation), with a few chunks' reductions offloaded to ACT.
All stages are software-pipelined across pairs so PE never waits on the
softmax chain.  Wf_b is dropped: softmax is shift-invariant.
"""

import sys
from contextlib import ExitStack

import ml_dtypes
import numpy as np

if "/opt/trn_rl_repo" not in sys.path:
    sys.path.insert(0, "/opt/trn_rl_repo")

import concourse.bass as bass  # noqa: F401  (bass types via bacc)
import concourse.tile as tile
from concourse import bacc, mybir
from concourse.bass import ts
from concourse.bass_utils import run_bass_kernel_spmd

N_CORES = 8
B, P, E, D, A = 256, 196, 2048, 512, 512
BL = B // N_CORES          # 32 batches per core
KO = E // 128              # 16 e-chunks
AC = A // 128              # 4 a-chunks
DC = D // 128              # 4 d-chunks
W = 2 * P                  # 392: moving free dim for a batch-pair
NPAIR = BL // 2            # 16
ACT_KOS = 2                # e-chunks whose awe-reduce runs on ACT

F32 = mybir.dt.float32
BF16 = mybir.dt.bfloat16
AF = mybir.ActivationFunctionType
OP = mybir.AluOpType


def build_kernel(repeat=1):
    nc = bacc.Bacc(
        "TRN2",
        target_bir_lowering=False,
        debug=False,
        enable_asserts=False,
        num_devices=N_CORES,
    )
    enc_t = nc.dram_tensor("enc_t", (NPAIR, E, 2, P), BF16, kind="ExternalInput").ap()
    dh_t = nc.dram_tensor("dh_t", (D, BL), BF16, kind="ExternalInput").ap()
    we = nc.dram_tensor("we", (E, A), BF16, kind="ExternalInput").ap()
    wd = nc.dram_tensor("wd", (D, A), BF16, kind="ExternalInput").ap()
    web = nc.dram_tensor("web", (A,), F32, kind="ExternalInput").ap()
    wdb = nc.dram_tensor("wdb", (A,), F32, kind="ExternalInput").ap()
    wf = nc.dram_tensor("wf", (A,), BF16, kind="ExternalInput").ap()
    awe_t = nc.dram_tensor("awe_t", (E, BL), F32, kind="ExternalOutput").ap()
    alpha_o = nc.dram_tensor("alpha_o", (1, BL, P), F32, kind="ExternalOutput").ap()

    with tile.TileContext(nc) as tc, ExitStack() as ctx:
        _emit(ctx, tc, enc_t, dh_t, we, wd, web, wdb, wf, awe_t, alpha_o, repeat)
    nc.compile()
    return nc


def _emit(ctx, tc, enc_t, dh_t, we, wd, web, wdb, wf, awe_t, alpha_o, repeat=1):
    nc = tc.nc

    cpool = ctx.enter_context(tc.tile_pool(name="consts", bufs=1))
    encb_pool = ctx.enter_context(tc.tile_pool(name="encb", bufs=4))
    h_pool = ctx.enter_context(tc.tile_pool(name="h", bufs=2))
    ab_pool = ctx.enter_context(tc.tile_pool(name="ab", bufs=3))
    scr_pool = ctx.enter_context(tc.tile_pool(name="scr", bufs=4))
    sm_pool = ctx.enter_context(tc.tile_pool(name="sm", bufs=2))
    psA = ctx.enter_context(tc.tile_pool(name="psA", bufs=6, space="PSUM"))
    psF = ctx.enter_context(tc.tile_pool(name="psF", bufs=2, space="PSUM"))

    # ---- resident constants (loaded once; the big We load is interleaved
    # with the first pair's encoder DMAs inside the loop below) --------------
    wd_sb = cpool.tile([128, DC, A], BF16, tag="wd")
    nc.sync.dma_start(wd_sb[:], wd.rearrange("(k p) a -> p k a", p=128))
    dh_sb = cpool.tile([128, DC, BL], BF16, tag="dh")
    nc.sync.dma_start(dh_sb[:], dh_t.rearrange("(k p) b -> p k b", p=128))
    wf_sb = cpool.tile([128, AC], BF16, tag="wf")
    nc.sync.dma_start(wf_sb[:], wf.rearrange("(k p) -> p k", p=128))
    web_sb = cpool.tile([128, AC], F32, tag="web")
    nc.sync.dma_start(web_sb[:], web.rearrange("(k p) -> p k", p=128))
    wdb_sb = cpool.tile([128, AC], F32, tag="wdb")
    nc.sync.dma_start(wdb_sb[:], wdb.rearrange("(k p) -> p k", p=128))
    we_sb = cpool.tile([128, KO, A], BF16, tag="we")

    bias_sb = cpool.tile([128, AC], F32, tag="bias")
    nc.vector.tensor_add(bias_sb[:], web_sb[:], wdb_sb[:])

    alpha_sb = cpool.tile([1, BL, P], F32, tag="alpha")
    awe_sb = cpool.tile([128, KO, BL], F32, tag="awe")
    att12_sb = cpool.tile([128, AC, BL], F32, tag="att12")

    # ---- att2 = dh @ Wd + (Wd_b + We_b), laid out [a_part, a_chunk, b] -----
    for ac in range(AC):
        ps2 = psA.tile([128, W], F32, tag="att1")
        for kd in range(DC):
            nc.tensor.matmul(
                ps2[:, :BL],
                lhsT=wd_sb[:, kd, ts(ac, 128)],
                rhs=dh_sb[:, kd, :],
                start=(kd == 0),
                stop=(kd == DC - 1),
            )
        nc.vector.tensor_tensor(
            att12_sb[:, ac, :],
            ps2[:, :BL],
            bias_sb[:, ac, None].to_broadcast((128, BL)),
            OP.add,
        )

    abs_ = {}

    def tail_head(state):
        """f = Wf . h ; softmax -> alpha_sb; broadcast alpha (pair i)."""
        i, encB, hT = state
        fp = psF.tile([1, W], F32, tag="f")
        for ac in range(AC):
            nc.tensor.matmul(
                fp[:],
                lhsT=wf_sb[:, ac : ac + 1],
                rhs=hT[:, ac, :],
                start=(ac == 0),
                stop=(ac == AC - 1),
            )
        nfmax = sm_pool.tile([1, 2], F32, tag="fmax")
        nc.vector.tensor_reduce(
            nfmax[:],
            fp.rearrange("o (b q) -> o b q", b=2),
            mybir.AxisListType.X,
            OP.max,
            negate=True,
        )
        expf = sm_pool.tile([1, 2, P], F32, tag="expf")
        sume = sm_pool.tile([1, 2], F32, tag="sume")
        for j in range(2):
            nc.scalar.activation(
                expf[:, j, :],
                fp[:, j * P : (j + 1) * P],
                AF.Exp,
                bias=nfmax[:, j : j + 1],
                scale=1.0,
                accum_out=sume[:, j : j + 1],
            )
        rsum = sm_pool.tile([1, 2], F32, tag="rsum")
        nc.vector.reciprocal(rsum[:], sume[:])
        for j in range(2):
            nc.vector.tensor_scalar_mul(
                alpha_sb[:, 2 * i + j, :], expf[:, j, :], rsum[:, j : j + 1]
            )
        alb = sm_pool.tile([1, W], BF16, tag="alb")
        nc.vector.tensor_copy(
            alb[:],
            alpha_sb[:, 2 * i : 2 * i + 2, :].rearrange("o b q -> o (b q)"),
        )
        ab = ab_pool.tile([128, W], BF16, tag="ab")
        nc.gpsimd.partition_broadcast(ab[:], alb[:])
        abs_[i] = ab

    def tail_awe(state, act_kos=ACT_KOS, last=False):
        """awe[:, :, pair] = sum_p encB * alpha (bf16, fp32 accumulation).

        DVE does the multiplies (and most reductions); act_kos chunks'
        reductions go to ACT.  For the flush pair, half the multiplies run
        on the otherwise-idle GPSIMD to shorten the drain."""
        i, encB, hT = state
        ab = abs_.pop(i)
        for ko in range(KO):
            tmp = scr_pool.tile([128, W], BF16, tag="tmp")
            mul_eng = nc.gpsimd if (last and ko % 2 == 1) else nc.vector
            mul_eng.tensor_tensor(tmp[:], encB[:, ko, :], ab[:], OP.mult)
            if ko < act_kos:
                for j in range(2):
                    scr = scr_pool.tile([128, P], BF16, tag="scr")
                    nc.scalar.activation(
                        scr[:],
                        tmp[:, j * P : (j + 1) * P],
                        AF.Copy,
                        bias=0.0,
                        scale=1.0,
                        accum_out=awe_sb[:, ko, 2 * i + j : 2 * i + j + 1],
                    )
            else:
                nc.vector.tensor_reduce(
                    awe_sb[:, ko, 2 * i : 2 * i + 2],
                    tmp.rearrange("p (b q) -> p b q", b=2),
                    mybir.AxisListType.X,
                    OP.add,
                )

    for _rep in range(repeat):
        prev = None
        for i in range(NPAIR):
            encB = encb_pool.tile([128, KO, W], BF16, tag="encB")
            for q in range(4):
                nc.sync.dma_start(
                    encB[:, ts(q, 4), :],
                    enc_t[i, ts(q, 512), :, :].rearrange(
                        "(k p) j q -> p k (j q)", p=128
                    ),
                )
                if _rep == 0 and i == 0:
                    nc.sync.dma_start(
                        we_sb[:, ts(q, 4), :],
                        we[ts(q, 512), :].rearrange("(k p) a -> p k a", p=128),
                    )
            if prev is not None:
                tail_head(prev)

            hT = h_pool.tile([128, AC, W], BF16, tag="h")
            for ac in range(AC):
                ps = psA.tile([128, W], F32, tag="att1")
                for ko in range(KO):
                    nc.tensor.matmul(
                        ps[:],
                        lhsT=we_sb[:, ko, ts(ac, 128)],
                        rhs=encB[:, ko, :],
                        start=(ko == 0),
                        stop=(ko == KO - 1),
                    )
                for j in range(2):
                    nc.scalar.activation(
                        hT[:, ac, j * P : (j + 1) * P],
                        ps[:, j * P : (j + 1) * P],
                        AF.Relu,
                        bias=att12_sb[:, ac, 2 * i + j : 2 * i + j + 1],
                        scale=1.0,
                    )
            if prev is not None:
                tail_awe(prev)
            prev = (i, encB, hT)

        tail_head(prev)
        tail_awe(prev, act_kos=10, last=True)

        # ---- outputs -------------------------------------------------------
        nc.sync.dma_start(awe_t.rearrange("(k p) b -> p k b", p=128), awe_sb[:])
        nc.sync.dma_start(alpha_o, alpha_sb[:])


_NC_CACHE = {}


def kernel(encoder_out, decoder_hidden, We_w, We_b, Wd_w, Wd_b, Wf_w, Wf_b):
    encoder_out = np.asarray(encoder_out, dtype=np.float32)
    decoder_hidden = np.asarray(decoder_hidden, dtype=np.float32)
    bf = ml_dtypes.bfloat16
    We_bf = np.ascontiguousarray(np.asarray(We_w, dtype=np.float32).astype(bf))
    Wd_bf = np.ascontiguousarray(np.asarray(Wd_w, dtype=np.float32).astype(bf))
    Wf_bf = np.ascontiguousarray(np.asarray(Wf_w, dtype=np.float32).astype(bf))
    We_b = np.ascontiguousarray(np.asarray(We_b, dtype=np.float32))
    Wd_b = np.ascontiguousarray(np.asarray(Wd_b, dtype=np.float32))
    del Wf_b  # softmax is shift-invariant; Wf_b cancels in both outputs

    if "nc" not in _NC_CACHE:
        _NC_CACHE["nc"] = build_kernel()
    nc = _NC_CACHE["nc"]

    in_maps = []
    for c in range(N_CORES):
        sl = slice(c * BL, (c + 1) * BL)
        enc_tp = np.ascontiguousarray(
            encoder_out[sl]
            .reshape(NPAIR, 2, P, E)
            .transpose(0, 3, 1, 2)
            .astype(bf)
        )
        in_maps.append(
            {
                "enc_t": enc_tp,
                "dh_t": np.ascontiguousarray(decoder_hidden[0, sl].T.astype(bf)),
                "we": We_bf,
                "wd": Wd_bf,
                "web": We_b,
                "wdb": Wd_b,
                "wf": Wf_bf,
            }
        )

    res = run_bass_kernel_spmd(nc, in_maps, core_ids=list(range(N_CORES)))
    awe = np.concatenate([r["awe_t"].T for r in res.results], axis=0)
    alpha = np.concatenate([r["alpha_o"][0] for r in res.results], axis=0)
    return awe, alpha


# revision 18
# speedup vs baseline: 1.0938x; 1.0329x over previous
"""Bahdanau-style attention kernel for 8 Trainium2 NeuronCores.

Computes (per reference):
    att1 = enc @ We + We_b          (B, P, A)
    att2 = dh @ Wd + Wd_b           (B, 1, A)
    h    = relu(att1 + att2)
    f    = h @ Wf + Wf_b            (B, P)
    alpha= softmax(f, axis=P)
    awe  = einsum('bpe,bp->be', enc, alpha)

Sharding: data-parallel over batch, 32 batches per core, weights replicated.
The host pre-transposes each encoder shard to (pair, E, 2, P) bf16 so that E
lands on SBUF partitions (contraction dim of the big matmul) and each DMA row
is 784B contiguous.  Batches are processed in pairs (moving free dim 392).

Per pair: PE accumulates att1 over 16 E-chunks in PSUM; ACT applies
relu(att1 + att2 + biases) with a per-partition bias; PE computes the logits
with a k-partitioned matvec; softmax runs on DVE/ACT (Exp with fused
accumulation); alpha is broadcast across partitions by GPSIMD; and
awe = sum_p alpha * enc runs on DVE (bf16 multiply + reduce, fp32
accumulation), with a few chunks' reductions offloaded to ACT.
All stages are software-pipelined across pairs so PE never waits on the
softmax chain.  Wf_b is dropped: softmax is shift-invariant.
"""

import sys
from contextlib import ExitStack

import ml_dtypes
import numpy as np

if "/opt/trn_rl_repo" not in sys.path:
    sys.path.insert(0, "/opt/trn_rl_repo")

import concourse.bass as bass  # noqa: F401  (bass types via bacc)
import concourse.tile as tile
from concourse import bacc, mybir
from concourse.bass import ts
from concourse.bass_utils import run_bass_kernel_spmd

N_CORES = 8
B, P, E, D, A = 256, 196, 2048, 512, 512
BL = B // N_CORES          # 32 batches per core
KO = E // 128              # 16 e-chunks
AC = A // 128              # 4 a-chunks
DC = D // 128              # 4 d-chunks
W = 2 * P                  # 392: moving free dim for a batch-pair
NPAIR = BL // 2            # 16
ACT_KOS = 2                # e-chunks whose awe-reduce runs on ACT

F32 = mybir.dt.float32
BF16 = mybir.dt.bfloat16
AF = mybir.ActivationFunctionType
OP = mybir.AluOpType


def build_kernel(repeat=1):
    nc = bacc.Bacc(
        "TRN2",
        target_bir_lowering=False,
        debug=False,
        enable_asserts=False,
        num_devices=N_CORES,
    )
    enc_t = nc.dram_tensor("enc_t", (NPAIR, E, 2, P), BF16, kind="ExternalInput").ap()
    dh_t = nc.dram_tensor("dh_t", (D, BL), BF16, kind="ExternalInput").ap()
    we = nc.dram_tensor("we", (E, A), BF16, kind="ExternalInput").ap()
    wd = nc.dram_tensor("wd", (D, A), BF16, kind="ExternalInput").ap()
    web = nc.dram_tensor("web", (A,), F32, kind="ExternalInput").ap()
    wdb = nc.dram_tensor("wdb", (A,), F32, kind="ExternalInput").ap()
    wf = nc.dram_tensor("wf", (A,), BF16, kind="ExternalInput").ap()
    awe_t = nc.dram_tensor("awe_t", (E, BL), F32, kind="ExternalOutput").ap()
    alpha_o = nc.dram_tensor("alpha_o", (1, BL, P), F32, kind="ExternalOutput").ap()

    with tile.TileContext(nc) as tc, ExitStack() as ctx:
        _emit(ctx, tc, enc_t, dh_t, we, wd, web, wdb, wf, awe_t, alpha_o, repeat)
    nc.compile()
    return nc


def _emit(ctx, tc, enc_t, dh_t, we, wd, web, wdb, wf, awe_t, alpha_o, repeat=1):
    nc = tc.nc

    cpool = ctx.enter_context(tc.tile_pool(name="consts", bufs=1))
    encb_pool = ctx.enter_context(tc.tile_pool(name="encb", bufs=5))
    h_pool = ctx.enter_context(tc.tile_pool(name="h", bufs=3))
    ab_pool = ctx.enter_context(tc.tile_pool(name="ab", bufs=4))
    scr_pool = ctx.enter_context(tc.tile_pool(name="scr", bufs=6))
    sm_pool = ctx.enter_context(tc.tile_pool(name="sm", bufs=3))
    psA = ctx.enter_context(tc.tile_pool(name="psA", bufs=6, space="PSUM"))
    psF = ctx.enter_context(tc.tile_pool(name="psF", bufs=2, space="PSUM"))

    # ---- resident constants (loaded once; the big We load is interleaved
    # with the first pair's encoder DMAs inside the loop below) --------------
    wd_sb = cpool.tile([128, DC, A], BF16, tag="wd")
    nc.sync.dma_start(wd_sb[:], wd.rearrange("(k p) a -> p k a", p=128))
    dh_sb = cpool.tile([128, DC, BL], BF16, tag="dh")
    nc.sync.dma_start(dh_sb[:], dh_t.rearrange("(k p) b -> p k b", p=128))
    wf_sb = cpool.tile([128, AC], BF16, tag="wf")
    nc.sync.dma_start(wf_sb[:], wf.rearrange("(k p) -> p k", p=128))
    web_sb = cpool.tile([128, AC], F32, tag="web")
    nc.sync.dma_start(web_sb[:], web.rearrange("(k p) -> p k", p=128))
    wdb_sb = cpool.tile([128, AC], F32, tag="wdb")
    nc.sync.dma_start(wdb_sb[:], wdb.rearrange("(k p) -> p k", p=128))
    we_sb = cpool.tile([128, KO, A], BF16, tag="we")

    bias_sb = cpool.tile([128, AC], F32, tag="bias")
    nc.vector.tensor_add(bias_sb[:], web_sb[:], wdb_sb[:])

    alpha_sb = cpool.tile([1, BL, P], F32, tag="alpha")
    awe_sb = cpool.tile([128, KO, BL], F32, tag="awe")
    att12_sb = cpool.tile([128, AC, BL], F32, tag="att12")

    # ---- att2 = dh @ Wd + (Wd_b + We_b), laid out [a_part, a_chunk, b] -----
    for ac in range(AC):
        ps2 = psA.tile([128, W], F32, tag="att1")
        for kd in range(DC):
            nc.tensor.matmul(
                ps2[:, :BL],
                lhsT=wd_sb[:, kd, ts(ac, 128)],
                rhs=dh_sb[:, kd, :],
                start=(kd == 0),
                stop=(kd == DC - 1),
            )
        nc.vector.tensor_tensor(
            att12_sb[:, ac, :],
            ps2[:, :BL],
            bias_sb[:, ac, None].to_broadcast((128, BL)),
            OP.add,
        )

    abs_ = {}

    def tail_head(state):
        """f = Wf . h ; softmax -> alpha_sb; broadcast alpha (pair i)."""
        i, encB, hT = state
        fp = psF.tile([1, W], F32, tag="f")
        for ac in range(AC):
            nc.tensor.matmul(
                fp[:],
                lhsT=wf_sb[:, ac : ac + 1],
                rhs=hT[:, ac, :],
                start=(ac == 0),
                stop=(ac == AC - 1),
            )
        nfmax = sm_pool.tile([1, 2], F32, tag="fmax")
        nc.vector.tensor_reduce(
            nfmax[:],
            fp.rearrange("o (b q) -> o b q", b=2),
            mybir.AxisListType.X,
            OP.max,
            negate=True,
        )
        expf = sm_pool.tile([1, 2, P], F32, tag="expf")
        sume = sm_pool.tile([1, 2], F32, tag="sume")
        for j in range(2):
            nc.scalar.activation(
                expf[:, j, :],
                fp[:, j * P : (j + 1) * P],
                AF.Exp,
                bias=nfmax[:, j : j + 1],
                scale=1.0,
                accum_out=sume[:, j : j + 1],
            )
        rsum = sm_pool.tile([1, 2], F32, tag="rsum")
        nc.vector.reciprocal(rsum[:], sume[:])
        for j in range(2):
            nc.vector.tensor_scalar_mul(
                alpha_sb[:, 2 * i + j, :], expf[:, j, :], rsum[:, j : j + 1]
            )
        alb = sm_pool.tile([1, W], BF16, tag="alb")
        nc.vector.tensor_copy(
            alb[:],
            alpha_sb[:, 2 * i : 2 * i + 2, :].rearrange("o b q -> o (b q)"),
        )
        ab = ab_pool.tile([128, W], BF16, tag="ab")
        nc.gpsimd.partition_broadcast(ab[:], alb[:])
        abs_[i] = ab

    def tail_awe(state, act_kos=ACT_KOS, last=False):
        """awe[:, :, pair] = sum_p encB * alpha (bf16, fp32 accumulation).

        DVE does the multiplies (and most reductions); act_kos chunks'
        reductions go to ACT.  For the flush pair, half the multiplies run
        on the otherwise-idle GPSIMD to shorten the drain."""
        i, encB, hT = state
        ab = abs_.pop(i)
        for ko in range(KO):
            tmp = scr_pool.tile([128, W], BF16, tag="tmp")
            mul_eng = nc.gpsimd if (last and ko % 2 == 1) else nc.vector
            mul_eng.tensor_tensor(tmp[:], encB[:, ko, :], ab[:], OP.mult)
            if ko < act_kos:
                for j in range(2):
                    scr = scr_pool.tile([128, P], BF16, tag="scr")
                    nc.scalar.activation(
                        scr[:],
                        tmp[:, j * P : (j + 1) * P],
                        AF.Copy,
                        bias=0.0,
                        scale=1.0,
                        accum_out=awe_sb[:, ko, 2 * i + j : 2 * i + j + 1],
                    )
            else:
                nc.vector.tensor_reduce(
                    awe_sb[:, ko, 2 * i : 2 * i + 2],
                    tmp.rearrange("p (b q) -> p b q", b=2),
                    mybir.AxisListType.X,
                    OP.add,
                )

    for _rep in range(repeat):
        prev = None
        for i in range(NPAIR):
            encB = encb_pool.tile([128, KO, W], BF16, tag="encB")
            for q in range(4):
                nc.sync.dma_start(
                    encB[:, ts(q, 4), :],
                    enc_t[i, ts(q, 512), :, :].rearrange(
                        "(k p) j q -> p k (j q)", p=128
                    ),
                )
                if _rep == 0 and i == 0:
                    nc.sync.dma_start(
                        we_sb[:, ts(q, 4), :],
                        we[ts(q, 512), :].rearrange("(k p) a -> p k a", p=128),
                    )
            if prev is not None:
                tail_head(prev)

            hT = h_pool.tile([128, AC, W], BF16, tag="h")
            for ac in range(AC):
                ps = psA.tile([128, W], F32, tag="att1")
                for ko in range(KO):
                    nc.tensor.matmul(
                        ps[:],
                        lhsT=we_sb[:, ko, ts(ac, 128)],
                        rhs=encB[:, ko, :],
                        start=(ko == 0),
                        stop=(ko == KO - 1),
                    )
                for j in range(2):
                    nc.scalar.activation(
                        hT[:, ac, j * P : (j + 1) * P],
                        ps[:, j * P : (j + 1) * P],
                        AF.Relu,
                        bias=att12_sb[:, ac, 2 * i + j : 2 * i + j + 1],
                        scale=1.0,
                    )
            if prev is not None:
                tail_awe(prev)
            prev = (i, encB, hT)

        tail_head(prev)
        tail_awe(prev, act_kos=10, last=True)

        # ---- outputs -------------------------------------------------------
        nc.sync.dma_start(awe_t.rearrange("(k p) b -> p k b", p=128), awe_sb[:])
        nc.sync.dma_start(alpha_o, alpha_sb[:])


_NC_CACHE = {}


def kernel(encoder_out, decoder_hidden, We_w, We_b, Wd_w, Wd_b, Wf_w, Wf_b):
    encoder_out = np.asarray(encoder_out, dtype=np.float32)
    decoder_hidden = np.asarray(decoder_hidden, dtype=np.float32)
    bf = ml_dtypes.bfloat16
    We_bf = np.ascontiguousarray(np.asarray(We_w, dtype=np.float32).astype(bf))
    Wd_bf = np.ascontiguousarray(np.asarray(Wd_w, dtype=np.float32).astype(bf))
    Wf_bf = np.ascontiguousarray(np.asarray(Wf_w, dtype=np.float32).astype(bf))
    We_b = np.ascontiguousarray(np.asarray(We_b, dtype=np.float32))
    Wd_b = np.ascontiguousarray(np.asarray(Wd_b, dtype=np.float32))
    del Wf_b  # softmax is shift-invariant; Wf_b cancels in both outputs

    if "nc" not in _NC_CACHE:
        _NC_CACHE["nc"] = build_kernel()
    nc = _NC_CACHE["nc"]

    in_maps = []
    for c in range(N_CORES):
        sl = slice(c * BL, (c + 1) * BL)
        enc_tp = np.ascontiguousarray(
            encoder_out[sl]
            .reshape(NPAIR, 2, P, E)
            .transpose(0, 3, 1, 2)
            .astype(bf)
        )
        in_maps.append(
            {
                "enc_t": enc_tp,
                "dh_t": np.ascontiguousarray(decoder_hidden[0, sl].T.astype(bf)),
                "we": We_bf,
                "wd": Wd_bf,
                "web": We_b,
                "wdb": Wd_b,
                "wf": Wf_bf,
            }
        )

    res = run_bass_kernel_spmd(nc, in_maps, core_ids=list(range(N_CORES)))
    awe = np.concatenate([r["awe_t"].T for r in res.results], axis=0)
    alpha = np.concatenate([r["alpha_o"][0] for r in res.results], axis=0)
    return awe, alpha
